# revision 20
# baseline (speedup 1.0000x reference)
"""Sliding-window attention (B=2,T=2048,C=1024,H=16,HD=64,WINDOW=524) on 8 trn2 cores.

Sharding: sequence-parallel. Core k = b*4+c owns query rows [c*512,(c+1)*512) of
batch b and receives x rows [c*512-524, c*512+512) (zero-padded outside the
sequence) so all of its attention windows are local. No collectives.

Per-core device pipeline (everything transposed so contractions land on the
partition axis, all matmul operands bf16, f32 accumulation):
  qT = Wq^T x^T (only own 512 rows), kT = Wk^T x^T (all 1152 local rows),
  RoPE folded into elementwise cos/sin scaling:
     scores q'.k' == (q * cs_i) . (2k * cs_j)  with cs = [cos;sin] per head,
  V computed in natural (t, c) layout with a ones column appended,
  S^T blocks (kv on partitions, q on free dim) matmul'd into two bank-aligned
  PSUM super-tiles per head, one exp per super-tile (P in bf16), one band-mask
  multiply per head, y^T (+ softmax denominator) = [V|1]^T @ P in PSUM,
  batched reciprocal over all 16 head denominators, normalize, Wo^T y^T.
"""

import os
import sys

import numpy as np

for _p in ("/opt/trn_rl_repo",):
    if _p not in sys.path and os.path.isdir(_p):
        sys.path.insert(0, _p)

import ml_dtypes

import concourse.bacc as bacc
import concourse.bass as bass
import concourse.mybir as mybir
from concourse.bass_utils import run_bass_kernel_spmd
from concourse.tile import TileContext

F32 = mybir.dt.float32
BF16 = mybir.dt.bfloat16
NPBF16 = ml_dtypes.bfloat16

B, T, C, H, HD = 2, 2048, 1024, 16, 64
WINDOW = 524
P = 128
CH = 512          # query rows per core
HALO = 524        # kv halo rows before the chunk
KT = 1040         # padded local kv length (1036 -> 1040; last j-chunk is 16 rows)
NJ = 9            # j-chunks: 8 full + one 16-row tail
NCC = C // P      # 8 contraction chunks
NCORE = 8

# Per-j-chunk query windows [lo, hi) in local query coords, 16-aligned lo.
JW = []
for _jc in range(NJ):
    _lo = max(0, P * _jc - 528)
    _hi = min(CH, P * _jc + P)
    JW.append((_lo, _hi))

# PSUM super-tile packing: each jc's scores window is placed bank-aligned
# (512-f32 banks) inside group G0 (3 banks) or G1 (3 banks).  pcol = column in
# the concatenated P/mask buffer [G0 | G1].
G0_OFF = {3: 0, 4: 512, 5: 1024}
G0_W = 1424
G1_OFF = {0: 0, 1: 128, 2: 512, 6: 1024, 7: 1296, 8: 1440}
G1_W = 1536
PW = G0_W + G1_W  # 2960
PCOL = {jc: off for jc, off in G0_OFF.items()}
PCOL.update({jc: G0_W + off for jc, off in G1_OFF.items()})

# k spans for the kT projection (rhs free width, psum bank limit 512)
KSPANS = [(0, 512), (512, 784), (784, 1040)]

JC_ORDER = [3, 4, 5, 0, 1, 2, 6, 7, 8]  # jc=3 first: its AV matmul covers [0,512)


def build_nc():
    nc = bacc.Bacc(None, target_bir_lowering=False)

    xT_d = nc.declare_dram_parameter("xT", [C, KT], BF16, isOutput=False)
    wqr_d = nc.declare_dram_parameter("wqr", [NCC, P, NCC, P], BF16, isOutput=False)
    wkr_d = nc.declare_dram_parameter("wkr", [NCC, P, NCC, P], BF16, isOutput=False)
    wor_d = nc.declare_dram_parameter("wor", [NCC, P, NCC, P], BF16, isOutput=False)
    wvr_d = nc.declare_dram_parameter("wvr", [NCC, P, C], BF16, isOutput=False)
    csq_d = nc.declare_dram_parameter("csq", [P, CH], F32, isOutput=False)
    csk_d = nc.declare_dram_parameter("csk", [P, KT], F32, isOutput=False)
    mask_d = nc.declare_dram_parameter("mask", [P, PW], BF16, isOutput=False)
    out_d = nc.declare_dram_parameter("out", [C, CH], F32, isOutput=True)

    Exp = mybir.ActivationFunctionType.Exp

    with TileContext(nc) as tc:
        with tc.tile_pool(name="persist", bufs=1) as pers:
            csq_sb = pers.tile([P, CH], F32, name="csq_sb")
            csk_sb = pers.tile([P, KT], F32, name="csk_sb")
            mask_sb = pers.tile([P, PW], BF16, name="mask_sb")

            qt_sb = []
            kt_sb = []
            yt_sb = []
            for cc2 in range(NCC):
                qt_sb.append(pers.tile([P, CH], BF16, name=f"qt{cc2}"))
                kt_sb.append(pers.tile([P, KT], BF16, name=f"kt{cc2}"))
                yt_sb.append(pers.tile([P, CH], BF16, name=f"yt{cc2}"))
            v_sb = pers.tile([P, NJ, H, 65], BF16, name="v_sb")
            nc.vector.memset(v_sb[:, :, :, 64], 1.0)
            wo_pre = []
            for cc2 in range(NCC):
                wot = pers.tile([P, NCC, P], BF16, name=f"wot{cc2}")
                wo_pre.append(wot)
            warm_a = pers.tile([P, P], BF16, name="warm_a")
            warm_b = pers.tile([P, CH], BF16, name="warm_b")
            nc.vector.memset(warm_a[:], 0.0)
            nc.vector.memset(warm_b[:], 0.0)

            # ---------------- Phase B1: warmup + V projection ----------------
            with (
                tc.tile_pool(name="xw", bufs=1) as xw,
                tc.tile_pool(name="wv_pool", bufs=1) as wvp,
            ):
                xT_sb = []
                wv_sb = []
                for cc in range(NCC):
                    xt = xw.tile([P, KT], BF16, name=f"xt{cc}")
                    nc.sync.dma_start(xt[:], xT_d[cc * P:(cc + 1) * P, :])
                    xT_sb.append(xt)
                    wvt = wvp.tile([P, C], BF16, name=f"wvt{cc}")
                    nc.sync.dma_start(wvt[:], wvr_d[cc])
                    wv_sb.append(wvt)
                nc.sync.dma_start(csq_sb[:], csq_d[:])
                nc.sync.dma_start(csk_sb[:], csk_d[:])
                nc.sync.dma_start(mask_sb[:], mask_d[:])
                for cc2 in range(NCC):
                    nc.sync.dma_start(wo_pre[cc2][:], wor_d[cc2])

                with (
                    tc.tile_pool(name="psv_pool", bufs=2, space="PSUM") as psvp,
                    tc.tile_pool(name="warm_pool", bufs=1, space="PSUM") as warmp,
                ):
                    ps_warm = warmp.tile([P, CH], F32, name="ps_warm")
                    for _ in range(56):
                        nc.tensor.matmul(ps_warm[:], lhsT=warm_a[:], rhs=warm_b[:],
                                         start=True, stop=True)

                    for tb in range(NJ):
                        tp = min(P, KT - tb * P)
                        for half in range(2):
                            psv = psvp.tile([P, 512], F32, name="psv")
                            for cc in range(NCC):
                                nc.tensor.matmul(
                                    psv[0:tp, :],
                                    lhsT=xT_sb[cc][:, tb * P:tb * P + tp],
                                    rhs=wv_sb[cc][:, half * 512:(half + 1) * 512],
                                    start=(cc == 0), stop=(cc == NCC - 1),
                                )
                            nc.scalar.copy(
                                v_sb[0:tp, tb, half * 8:(half + 1) * 8, 0:64],
                                psv[0:tp, :].rearrange("p (h d) -> p h d", h=8),
                            )

                # ------- Phase B2/C: software-pipelined projections + attention -------
                with (
                    tc.tile_pool(name="ws", bufs=3) as ws,
                    tc.tile_pool(name="ppool", bufs=4) as ppool,
                    tc.tile_pool(name="rpool", bufs=3) as rpool,
                    tc.tile_pool(name="pq_pool", bufs=1, space="PSUM") as pqp,
                    tc.tile_pool(name="pst_pool", bufs=1, space="PSUM") as pstp,
                    tc.tile_pool(name="pay_pool", bufs=1, space="PSUM") as payp,
                ):
                    wqk_t = {}
                    P_tiles = {}
                    psg = {}

                    def dma_w(s):
                        wq_t = ws.tile([P, NCC, P], BF16, name="wq_t")
                        nc.sync.dma_start(wq_t[:], wqr_d[s])
                        wk_t = ws.tile([P, NCC, P], BF16, name="wk_t")
                        nc.sync.dma_start(wk_t[:], wkr_d[s])
                        wqk_t[s] = (wq_t, wk_t)

                    def q_group(s):
                        wq_t = wqk_t[s][0]
                        psq = pqp.tile([P, CH], F32, name="psq")
                        for cc in range(NCC):
                            nc.tensor.matmul(
                                psq[:], lhsT=wq_t[:, cc, :],
                                rhs=xT_sb[cc][:, HALO:HALO + CH],
                                start=(cc == 0), stop=(cc == NCC - 1),
                            )
                        nc.vector.tensor_mul(qt_sb[s][:], psq[:], csq_sb[:])

                    def k_group(s, ki):
                        wk_t = wqk_t[s][1]
                        a, b = KSPANS[ki]
                        psk = pqp.tile([P, CH], F32, name="psq")
                        for cc in range(NCC):
                            nc.tensor.matmul(
                                psk[:, :b - a], lhsT=wk_t[:, cc, :],
                                rhs=xT_sb[cc][:, a:b],
                                start=(cc == 0), stop=(cc == NCC - 1),
                            )
                        nc.vector.tensor_mul(
                            kt_sb[s][:, a:b], psk[:, :b - a], csk_sb[:, a:b])

                    def st_exp_mask(h):
                        cc2, po = h // 2, (h % 2) * 64
                        ps_g0 = pstp.tile([P, G0_W], F32, name="ps_g0")
                        ps_g1 = pstp.tile([P, G1_W], F32, name="ps_g1")
                        P_t = ppool.tile([P, PW], BF16, name="P_t")
                        psg[h] = (ps_g0, ps_g1)
                        P_tiles[h] = P_t
                        for jc in JC_ORDER:
                            lo, hi = JW[jc]
                            w = hi - lo
                            jp = min(P, KT - jc * P)
                            if jc in G0_OFF:
                                dst = ps_g0[0:jp, G0_OFF[jc]:G0_OFF[jc] + w]
                            else:
                                dst = ps_g1[0:jp, G1_OFF[jc]:G1_OFF[jc] + w]
                            nc.tensor.matmul(
                                dst,
                                lhsT=kt_sb[cc2][po:po + 64, jc * P:jc * P + jp],
                                rhs=qt_sb[cc2][po:po + 64, lo:hi],
                                start=True, stop=True,
                            )
                        nc.scalar.activation(P_t[:, 0:G0_W], ps_g0[:], Exp,
                                             scale=0.125)
                        nc.scalar.activation(P_t[:, G0_W:PW], ps_g1[:], Exp,
                                             scale=0.125)
                        nc.vector.tensor_mul(P_t[:], P_t[:], mask_sb[:])

                    def av_norm(h):
                        cc2, po = h // 2, (h % 2) * 64
                        P_t = P_tiles.pop(h)
                        psg.pop(h)
                        ps_y = payp.tile([P, CH], F32, name="ps_y")
                        for idx, jc in enumerate(JC_ORDER):
                            lo, hi = JW[jc]
                            w = hi - lo
                            pc = PCOL[jc]
                            jp = min(P, KT - jc * P)
                            nc.tensor.matmul(
                                ps_y[0:65, lo:hi],
                                lhsT=v_sb[0:jp, jc, h, :],
                                rhs=P_t[0:jp, pc:pc + w],
                                start=(idx == 0), stop=(idx == NJ - 1),
                            )
                        dtmp = rpool.tile([1, CH], F32, name="dtmp")
                        nc.vector.tensor_copy(dtmp[:], ps_y[64:65, :])
                        rcp_row = rpool.tile([1, CH], F32, name="rcp_row")
                        nc.vector.reciprocal_approx_fast(rcp_row[:], dtmp[:])
                        rcp_bc = rpool.tile([P, CH], F32, name="rcp_bc")
                        nc.gpsimd.partition_broadcast(rcp_bc[:], rcp_row[:])
                        nc.vector.tensor_mul(
                            yt_sb[cc2][po:po + 64, :], ps_y[0:64, :],
                            rcp_bc[po:po + 64, :],
                        )

                    def warm_tail(n):
                        psd = pqp.tile([P, CH], F32, name="psq")
                        for _ in range(n):
                            nc.tensor.matmul(psd[:], lhsT=warm_a[:], rhs=warm_b[:],
                                             start=True, stop=True)

                    dma_w(0)
                    dma_w(1)
                    for s in range(10):
                        if s + 2 < NCC:
                            dma_w(s + 2)
                        hB = [2 * (s - 1), 2 * (s - 1) + 1] if 1 <= s <= 8 else []
                        hC = [2 * (s - 2), 2 * (s - 2) + 1] if 2 <= s <= 9 else []
                        if s < NCC:
                            q_group(s)
                        if hB:
                            st_exp_mask(hB[0])
                        if s < NCC:
                            k_group(s, 0)
                            k_group(s, 1)
                        if hB:
                            st_exp_mask(hB[1])
                        if s < NCC:
                            k_group(s, 2)
                        if s >= NCC:
                            warm_tail(10)
                        if hC:
                            av_norm(hC[0])
                            if s >= NCC:
                                warm_tail(6)
                            av_norm(hC[1])
                        if s >= NCC:
                            warm_tail(6)

            # ---------------- Phase D: output projection ----------------
            with (
                tc.tile_pool(name="obuf", bufs=2) as obuf,
                tc.tile_pool(name="po_pool", bufs=2, space="PSUM") as pop,
            ):
                for cc2 in range(NCC):
                    wo_t = wo_pre[cc2]
                    pso = pop.tile([P, CH], F32, name="pso")
                    for cc in range(NCC):
                        nc.tensor.matmul(
                            pso[:],
                            lhsT=wo_t[:, cc, :],
                            rhs=yt_sb[cc][:],
                            start=(cc == 0), stop=(cc == NCC - 1),
                        )
                    ob = obuf.tile([P, CH], F32, name="ob")
                    nc.scalar.copy(ob[:], pso[:])
                    nc.sync.dma_start(out_d[cc2 * P:(cc2 + 1) * P, :], ob[:])

    nc.compile()
    return nc


def _host_mask():
    """(128, PW) bf16 band mask per core; 0 in pack holes and outside band."""
    masks = []
    for core in range(NCORE):
        c = core % 4
        g0 = c * CH - HALO
        m = np.zeros((P, PW), np.float32)
        for jc in range(NJ):
            lo, hi = JW[jc]
            w = hi - lo
            j = (jc * P + np.arange(P))[:, None]          # local kv row
            i = (lo + np.arange(w))[None, :]              # local q row
            ok = (j >= i + 1) & (j <= i + WINDOW) & (g0 + j >= 0)
            pc = PCOL[jc]
            m[:, pc:pc + w] = ok.astype(np.float32)
        masks.append(m.astype(NPBF16))
    return masks


_MASKS = _host_mask()
_NC_CACHE = {}


def _get_nc():
    if "nc" not in _NC_CACHE:
        _NC_CACHE["nc"] = build_nc()
    return _NC_CACHE["nc"]


def _in_maps(x, Wq, Wk, Wv, Wo, rope_cos, rope_sin):
    x = np.asarray(x, np.float32)
    cos = np.asarray(rope_cos, np.float32)[:, 0, :]   # (T, 32)
    sin = np.asarray(rope_sin, np.float32)[:, 0, :]

    def wr_cols(W):
        # [cc2, p, cc, m] = W[cc*128+p, cc2*128+m], contiguous per cc2
        return np.ascontiguousarray(
            np.asarray(W, np.float32).reshape(NCC, P, NCC, P).transpose(2, 1, 0, 3)
        ).astype(NPBF16)

    wqr = wr_cols(Wq)
    wkr = wr_cols(Wk)
    wor = wr_cols(Wo)
    wvr = np.asarray(Wv, np.float32).reshape(NCC, P, C).astype(NPBF16)

    maps = []
    for core in range(NCORE):
        b, c = divmod(core, 4)
        s = c * CH
        g0 = s - HALO
        xs = np.zeros((KT, C), np.float32)
        a0 = max(0, g0)
        xs[a0 - g0:s + CH - g0] = x[b, a0:s + CH]
        xT = np.ascontiguousarray(xs.T).astype(NPBF16)

        csq = np.concatenate([cos[s:s + CH].T, sin[s:s + CH].T], 0)      # (64, 512)
        gidx = np.clip(g0 + np.arange(KT), 0, T - 1)
        csk = 2.0 * np.concatenate([cos[gidx].T, sin[gidx].T], 0)        # (64, KT)
        maps.append({
            "xT": xT,
            "wqr": wqr, "wkr": wkr, "wor": wor, "wvr": wvr,
            "csq": np.ascontiguousarray(np.tile(csq, (2, 1))),
            "csk": np.ascontiguousarray(np.tile(csk, (2, 1))),
            "mask": _MASKS[core],
        })
    return maps


def run(inputs, **kw):
    nc = _get_nc()
    maps = _in_maps(**inputs)
    res = run_bass_kernel_spmd(nc, maps, core_ids=list(range(NCORE)), **kw)
    out = np.zeros((B, T, C), np.float32)
    for core in range(NCORE):
        b, c = divmod(core, 4)
        s = c * CH
        out[b, s:s + CH, :] = res.results[core]["out"].T
    return out, res


def kernel(**inputs):
    out, _ = run(inputs)
    return out


if __name__ == "__main__":
    # graph-build smoke test
    nc = build_nc()
    print("build ok")


# revision 21
# speedup vs baseline: 1.0125x; 1.0125x over previous
"""Sliding-window attention (B=2,T=2048,C=1024,H=16,HD=64,WINDOW=524) on 8 trn2 cores.

Sharding: sequence-parallel. Core k = b*4+c owns query rows [c*512,(c+1)*512) of
batch b and receives x rows [c*512-524, c*512+512) (zero-padded outside the
sequence) so all of its attention windows are local. No collectives.

Per-core device pipeline (everything transposed so contractions land on the
partition axis, all matmul operands bf16, f32 accumulation):
  qT = Wq^T x^T (only own 512 rows), kT = Wk^T x^T (all 1152 local rows),
  RoPE folded into elementwise cos/sin scaling:
     scores q'.k' == (q * cs_i) . (2k * cs_j)  with cs = [cos;sin] per head,
  V computed in natural (t, c) layout with a ones column appended,
  S^T blocks (kv on partitions, q on free dim) matmul'd into two bank-aligned
  PSUM super-tiles per head, one exp per super-tile (P in bf16), one band-mask
  multiply per head, y^T (+ softmax denominator) = [V|1]^T @ P in PSUM,
  per-head fast-approx reciprocal of the denominator row, normalize, Wo^T y^T.
  The cc2 loop is software-pipelined: projection matmul groups for chunk s are
  woven between attention ST/AV matmuls of chunks s-1/s-2 so the TensorEngine
  stream stays dense and the HAM clock stays at 2.4 GHz; dummy warm matmuls
  bridge the DMA-bound prologue and the attention drain.
"""

import os
import sys

import numpy as np

for _p in ("/opt/trn_rl_repo",):
    if _p not in sys.path and os.path.isdir(_p):
        sys.path.insert(0, _p)

import ml_dtypes

import concourse.bacc as bacc
import concourse.bass as bass
import concourse.mybir as mybir
from concourse.bass_utils import run_bass_kernel_spmd
from concourse.tile import TileContext

F32 = mybir.dt.float32
BF16 = mybir.dt.bfloat16
NPBF16 = ml_dtypes.bfloat16

B, T, C, H, HD = 2, 2048, 1024, 16, 64
WINDOW = 524
P = 128
CH = 512          # query rows per core
HALO = 524        # kv halo rows before the chunk
KT = 1040         # padded local kv length (1036 -> 1040; last j-chunk is 16 rows)
NJ = 9            # j-chunks: 8 full + one 16-row tail
NCC = C // P      # 8 contraction chunks
NCORE = 8

# Per-j-chunk query windows [lo, hi) in local query coords, 16-aligned lo.
JW = []
for _jc in range(NJ):
    _lo = max(0, P * _jc - 528)
    _hi = min(CH, P * _jc + P)
    JW.append((_lo, _hi))

# PSUM super-tile packing: each jc's scores window is placed bank-aligned
# (512-f32 banks) inside group G0 (3 banks) or G1 (3 banks).  pcol = column in
# the concatenated P/mask buffer [G0 | G1].
G0_OFF = {3: 0, 4: 512, 5: 1024}
G0_W = 1424
G1_OFF = {0: 0, 1: 128, 2: 512, 6: 1024, 7: 1296, 8: 1440}
G1_W = 1536
PW = G0_W + G1_W  # 2960
PCOL = {jc: off for jc, off in G0_OFF.items()}
PCOL.update({jc: G0_W + off for jc, off in G1_OFF.items()})

# k spans for the kT projection (rhs free width, psum bank limit 512)
KSPANS = [(0, 512), (512, 784), (784, 1040)]

JC_ORDER = [3, 4, 5, 0, 1, 2, 6, 7, 8]  # jc=3 first: its AV matmul covers [0,512)


def build_nc():
    nc = bacc.Bacc(None, target_bir_lowering=False)

    xT_d = nc.declare_dram_parameter("xT", [C, KT], BF16, isOutput=False)
    wqr_d = nc.declare_dram_parameter("wqr", [NCC, P, NCC, P], BF16, isOutput=False)
    wkr_d = nc.declare_dram_parameter("wkr", [NCC, P, NCC, P], BF16, isOutput=False)
    wor_d = nc.declare_dram_parameter("wor", [NCC, P, NCC, P], BF16, isOutput=False)
    wvr_d = nc.declare_dram_parameter("wvr", [NCC, P, C], BF16, isOutput=False)
    csq_d = nc.declare_dram_parameter("csq", [P, CH], F32, isOutput=False)
    csk_d = nc.declare_dram_parameter("csk", [P, KT], F32, isOutput=False)
    mask_d = nc.declare_dram_parameter("mask", [P, PW], BF16, isOutput=False)
    out_d = nc.declare_dram_parameter("out", [C, CH], F32, isOutput=True)

    Exp = mybir.ActivationFunctionType.Exp

    with TileContext(nc) as tc:
        with tc.tile_pool(name="persist", bufs=1) as pers:
            csq_sb = pers.tile([P, CH], F32, name="csq_sb")
            csk_sb = pers.tile([P, KT], F32, name="csk_sb")
            mask_sb = pers.tile([P, PW], BF16, name="mask_sb")

            qt_sb = []
            kt_sb = []
            yt_sb = []
            for cc2 in range(NCC):
                qt_sb.append(pers.tile([P, CH], BF16, name=f"qt{cc2}"))
                kt_sb.append(pers.tile([P, KT], BF16, name=f"kt{cc2}"))
                yt_sb.append(pers.tile([P, CH], BF16, name=f"yt{cc2}"))
            v_sb = pers.tile([P, NJ, H, 65], BF16, name="v_sb")
            nc.vector.memset(v_sb[:, :, :, 64], 1.0)
            wo_pre = []
            for cc2 in range(NCC):
                wot = pers.tile([P, NCC, P], BF16, name=f"wot{cc2}")
                wo_pre.append(wot)
            warm_a = pers.tile([P, P], BF16, name="warm_a")
            warm_b = pers.tile([P, CH], BF16, name="warm_b")
            nc.vector.memset(warm_a[:], 0.0)
            nc.vector.memset(warm_b[:], 0.0)

            # ---------------- Phase B1: warmup + V projection ----------------
            with (
                tc.tile_pool(name="xw", bufs=1) as xw,
                tc.tile_pool(name="wv_pool", bufs=1) as wvp,
            ):
                xT_sb = []
                wv_sb = []
                for cc in range(NCC):
                    xt = xw.tile([P, KT], BF16, name=f"xt{cc}")
                    nc.sync.dma_start(xt[:], xT_d[cc * P:(cc + 1) * P, :])
                    xT_sb.append(xt)
                    wvt = wvp.tile([P, C], BF16, name=f"wvt{cc}")
                    nc.sync.dma_start(wvt[:], wvr_d[cc])
                    wv_sb.append(wvt)
                nc.sync.dma_start(csq_sb[:], csq_d[:])
                nc.sync.dma_start(csk_sb[:], csk_d[:])
                nc.sync.dma_start(mask_sb[:], mask_d[:])
                for cc2 in range(NCC):
                    nc.sync.dma_start(wo_pre[cc2][:], wor_d[cc2])

                with (
                    tc.tile_pool(name="psv_pool", bufs=2, space="PSUM") as psvp,
                    tc.tile_pool(name="warm_pool", bufs=1, space="PSUM") as warmp,
                ):
                    ps_warm = warmp.tile([P, CH], F32, name="ps_warm")
                    for _ in range(56):
                        nc.tensor.matmul(ps_warm[:], lhsT=warm_a[:], rhs=warm_b[:],
                                         start=True, stop=True)

                    for tb in range(NJ):
                        tp = min(P, KT - tb * P)
                        for half in range(2):
                            psv = psvp.tile([P, 512], F32, name="psv")
                            for cc in range(NCC):
                                nc.tensor.matmul(
                                    psv[0:tp, :],
                                    lhsT=xT_sb[cc][:, tb * P:tb * P + tp],
                                    rhs=wv_sb[cc][:, half * 512:(half + 1) * 512],
                                    start=(cc == 0), stop=(cc == NCC - 1),
                                )
                            nc.scalar.copy(
                                v_sb[0:tp, tb, half * 8:(half + 1) * 8, 0:64],
                                psv[0:tp, :].rearrange("p (h d) -> p h d", h=8),
                            )

                # ------- Phase B2/C: software-pipelined projections + attention -------
                with (
                    tc.tile_pool(name="ws", bufs=3) as ws,
                    tc.tile_pool(name="ppool", bufs=4) as ppool,
                    tc.tile_pool(name="rpool", bufs=3) as rpool,
                    tc.tile_pool(name="pq_pool", bufs=1, space="PSUM") as pqp,
                    tc.tile_pool(name="pst_pool", bufs=1, space="PSUM") as pstp,
                    tc.tile_pool(name="pay_pool", bufs=1, space="PSUM") as payp,
                ):
                    wqk_t = {}
                    P_tiles = {}
                    psg = {}

                    def dma_w(s):
                        wq_t = ws.tile([P, NCC, P], BF16, name="wq_t")
                        nc.sync.dma_start(wq_t[:], wqr_d[s])
                        wk_t = ws.tile([P, NCC, P], BF16, name="wk_t")
                        nc.sync.dma_start(wk_t[:], wkr_d[s])
                        wqk_t[s] = (wq_t, wk_t)

                    def q_group(s):
                        wq_t = wqk_t[s][0]
                        psq = pqp.tile([P, CH], F32, name="psq")
                        for cc in range(NCC):
                            nc.tensor.matmul(
                                psq[:], lhsT=wq_t[:, cc, :],
                                rhs=xT_sb[cc][:, HALO:HALO + CH],
                                start=(cc == 0), stop=(cc == NCC - 1),
                            )
                        nc.vector.tensor_mul(qt_sb[s][:], psq[:], csq_sb[:])

                    def k_group(s, ki):
                        wk_t = wqk_t[s][1]
                        a, b = KSPANS[ki]
                        psk = pqp.tile([P, CH], F32, name="psq")
                        for cc in range(NCC):
                            nc.tensor.matmul(
                                psk[:, :b - a], lhsT=wk_t[:, cc, :],
                                rhs=xT_sb[cc][:, a:b],
                                start=(cc == 0), stop=(cc == NCC - 1),
                            )
                        nc.vector.tensor_mul(
                            kt_sb[s][:, a:b], psk[:, :b - a], csk_sb[:, a:b])

                    def st_exp_mask(h):
                        cc2, po = h // 2, (h % 2) * 64
                        ps_g0 = pstp.tile([P, G0_W], F32, name="ps_g0")
                        ps_g1 = pstp.tile([P, G1_W], F32, name="ps_g1")
                        P_t = ppool.tile([P, PW], BF16, name="P_t")
                        psg[h] = (ps_g0, ps_g1)
                        P_tiles[h] = P_t
                        for jc in JC_ORDER:
                            lo, hi = JW[jc]
                            w = hi - lo
                            jp = min(P, KT - jc * P)
                            if jc in G0_OFF:
                                dst = ps_g0[0:jp, G0_OFF[jc]:G0_OFF[jc] + w]
                            else:
                                dst = ps_g1[0:jp, G1_OFF[jc]:G1_OFF[jc] + w]
                            nc.tensor.matmul(
                                dst,
                                lhsT=kt_sb[cc2][po:po + 64, jc * P:jc * P + jp],
                                rhs=qt_sb[cc2][po:po + 64, lo:hi],
                                start=True, stop=True,
                            )
                        nc.scalar.activation(P_t[:, 0:G0_W], ps_g0[:], Exp,
                                             scale=0.125)
                        nc.scalar.activation(P_t[:, G0_W:PW], ps_g1[:], Exp,
                                             scale=0.125)
                        nc.vector.tensor_mul(P_t[:], P_t[:], mask_sb[:])

                    def av_norm(h):
                        cc2, po = h // 2, (h % 2) * 64
                        P_t = P_tiles.pop(h)
                        psg.pop(h)
                        ps_y = payp.tile([P, CH], F32, name="ps_y")
                        for idx, jc in enumerate(JC_ORDER):
                            lo, hi = JW[jc]
                            w = hi - lo
                            pc = PCOL[jc]
                            jp = min(P, KT - jc * P)
                            nc.tensor.matmul(
                                ps_y[0:65, lo:hi],
                                lhsT=v_sb[0:jp, jc, h, :],
                                rhs=P_t[0:jp, pc:pc + w],
                                start=(idx == 0), stop=(idx == NJ - 1),
                            )
                        dtmp = rpool.tile([1, CH], F32, name="dtmp")
                        nc.vector.tensor_copy(dtmp[:], ps_y[64:65, :])
                        rcp_row = rpool.tile([1, CH], F32, name="rcp_row")
                        nc.vector.reciprocal_approx_fast(rcp_row[:], dtmp[:])
                        rcp_bc = rpool.tile([P, CH], F32, name="rcp_bc")
                        nc.gpsimd.partition_broadcast(rcp_bc[:], rcp_row[:])
                        nc.vector.tensor_mul(
                            yt_sb[cc2][po:po + 64, :], ps_y[0:64, :],
                            rcp_bc[po:po + 64, :],
                        )

                    def warm_tail(n):
                        psd = pqp.tile([P, CH], F32, name="psq")
                        for _ in range(n):
                            nc.tensor.matmul(psd[:], lhsT=warm_a[:], rhs=warm_b[:],
                                             start=True, stop=True)

                    dma_w(0)
                    dma_w(1)
                    for s in range(10):
                        if s + 2 < NCC:
                            dma_w(s + 2)
                        hB = [2 * (s - 1), 2 * (s - 1) + 1] if 1 <= s <= 8 else []
                        hC = [2 * (s - 2), 2 * (s - 2) + 1] if 2 <= s <= 9 else []
                        if s < NCC:
                            q_group(s)
                        if hB:
                            st_exp_mask(hB[0])
                        if s < NCC:
                            k_group(s, 0)
                            k_group(s, 1)
                        if hB:
                            st_exp_mask(hB[1])
                        if s < NCC:
                            k_group(s, 2)
                        if s >= NCC:
                            warm_tail(6)
                        if hC:
                            av_norm(hC[0])
                            av_norm(hC[1])
                        if s >= NCC:
                            warm_tail(6)

            # ---------------- Phase D: output projection ----------------
            with (
                tc.tile_pool(name="obuf", bufs=2) as obuf,
                tc.tile_pool(name="po_pool", bufs=2, space="PSUM") as pop,
            ):
                for cc2 in range(NCC):
                    wo_t = wo_pre[cc2]
                    pso = pop.tile([P, CH], F32, name="pso")
                    for cc in range(NCC):
                        nc.tensor.matmul(
                            pso[:],
                            lhsT=wo_t[:, cc, :],
                            rhs=yt_sb[cc][:],
                            start=(cc == 0), stop=(cc == NCC - 1),
                        )
                    ob = obuf.tile([P, CH], F32, name="ob")
                    nc.scalar.copy(ob[:], pso[:])
                    nc.sync.dma_start(out_d[cc2 * P:(cc2 + 1) * P, :], ob[:])

    nc.compile()
    return nc


def _host_mask():
    """(128, PW) bf16 band mask per core; 0 in pack holes and outside band."""
    masks = []
    for core in range(NCORE):
        c = core % 4
        g0 = c * CH - HALO
        m = np.zeros((P, PW), np.float32)
        for jc in range(NJ):
            lo, hi = JW[jc]
            w = hi - lo
            j = (jc * P + np.arange(P))[:, None]          # local kv row
            i = (lo + np.arange(w))[None, :]              # local q row
            ok = (j >= i + 1) & (j <= i + WINDOW) & (g0 + j >= 0)
            pc = PCOL[jc]
            m[:, pc:pc + w] = ok.astype(np.float32)
        masks.append(m.astype(NPBF16))
    return masks


_MASKS = _host_mask()
_NC_CACHE = {}


def _get_nc():
    if "nc" not in _NC_CACHE:
        _NC_CACHE["nc"] = build_nc()
    return _NC_CACHE["nc"]


def _in_maps(x, Wq, Wk, Wv, Wo, rope_cos, rope_sin):
    x = np.asarray(x, np.float32)
    cos = np.asarray(rope_cos, np.float32)[:, 0, :]   # (T, 32)
    sin = np.asarray(rope_sin, np.float32)[:, 0, :]

    def wr_cols(W):
        # [cc2, p, cc, m] = W[cc*128+p, cc2*128+m], contiguous per cc2
        return np.ascontiguousarray(
            np.asarray(W, np.float32).reshape(NCC, P, NCC, P).transpose(2, 1, 0, 3)
        ).astype(NPBF16)

    wqr = wr_cols(Wq)
    wkr = wr_cols(Wk)
    wor = wr_cols(Wo)
    wvr = np.asarray(Wv, np.float32).reshape(NCC, P, C).astype(NPBF16)

    maps = []
    for core in range(NCORE):
        b, c = divmod(core, 4)
        s = c * CH
        g0 = s - HALO
        xs = np.zeros((KT, C), np.float32)
        a0 = max(0, g0)
        xs[a0 - g0:s + CH - g0] = x[b, a0:s + CH]
        xT = np.ascontiguousarray(xs.T).astype(NPBF16)

        csq = np.concatenate([cos[s:s + CH].T, sin[s:s + CH].T], 0)      # (64, 512)
        gidx = np.clip(g0 + np.arange(KT), 0, T - 1)
        csk = 2.0 * np.concatenate([cos[gidx].T, sin[gidx].T], 0)        # (64, KT)
        maps.append({
            "xT": xT,
            "wqr": wqr, "wkr": wkr, "wor": wor, "wvr": wvr,
            "csq": np.ascontiguousarray(np.tile(csq, (2, 1))),
            "csk": np.ascontiguousarray(np.tile(csk, (2, 1))),
            "mask": _MASKS[core],
        })
    return maps


def run(inputs, **kw):
    nc = _get_nc()
    maps = _in_maps(**inputs)
    res = run_bass_kernel_spmd(nc, maps, core_ids=list(range(NCORE)), **kw)
    out = np.zeros((B, T, C), np.float32)
    for core in range(NCORE):
        b, c = divmod(core, 4)
        s = c * CH
        out[b, s:s + CH, :] = res.results[core]["out"].T
    return out, res


def kernel(**inputs):
    out, _ = run(inputs)
    return out


if __name__ == "__main__":
    # graph-build smoke test
    nc = build_nc()
    print("build ok")
